# revision 17
# baseline (speedup 1.0000x reference)
"""Trainium2 Bass kernel for GQA attention with RoPE (dense transformer).

Problem: B=2, S=2048, H=2048, 16 query heads / 4 KV heads, head_dim 128,
causal flash-style attention, fused QKV + o_proj.

Sharding (8 cores): (batch, head-group) grid. Core c handles batch c//4 and
head group c%4 (4 query heads + their shared KV head). o_proj is computed as
per-group partials reduced on host (tensor-parallel o_proj input split).

v2 vs baseline (336us):
  - bf16 activations/weights end-to-end (PE rate is identical to f32r, but
    DMA bytes and DVE element throughput both improve 2x; accuracy measured
    ~2e-3 rel, threshold 2e-2). PSUM accumulation stays fp32.
  - Softmax denominators: per (h,chunk) the exp tiles are accumulated over
    k-tiles on the DVE, then ONE ones-matmul computes the partition sum
    (was: one ones-matmul per k-tile; -144 PE matmuls).
  - Batched DMA: x loads as 1MB half-chunks, weights as whole tensors,
    outputs as one 0.5MB DMA per 128-token tile. Weights go on the ACT
    HWDGE queue, x/out on the SP queue.
  - o_proj is emitted interleaved (right after each q-chunk's attention),
    not as a tail phase, so its matmuls fill PE gaps left by exp pacing
    and the output DMA is spread across the kernel.

On-core layout: activations live as [feature, token] ("transposed") so the
feature contraction dims land on SBUF partitions for the PE array.
Causal masking: fully-masked k-tiles are skipped entirely; diagonal tiles
get a zero-fill triangle (affine_select on GpSimd) after exp.
"""
import math

import numpy as np

import concourse.bass as bass
import concourse.mybir as mybir
import concourse.tile as tile
from concourse import bacc
from concourse.bass_utils import run_bass_kernel_spmd
from concourse.masks import make_identity

B, S, H = 2, 2048, 2048
NH, KVH, HD = 16, 4, 128
G = 4                 # head groups (= KVH); grid = G x B = 8 cores
GQ = NH // KVH        # query heads per group
QD = GQ * HD          # per-core q dim (512)
KC = H // 128         # contraction chunks for projections (16)
TC = 4                # token chunks of 512
TT = S // 128         # 128-token tiles (16)

F32 = mybir.dt.float32
BF = mybir.dt.bfloat16
AF = mybir.ActivationFunctionType

_NC = None


def _emit(nc):
    # All big inputs are host-packed partition-major: row p is the full
    # contiguous per-partition payload, so every DMA is 128 descriptors of
    # >=4KB regardless of logical shape (HWDGE issue cost is ~5ns/descriptor).
    # x is packed token-chunk-major: [p, t, ko, c] so chunk t's whole
    # projection input is one contiguous 16KB-per-partition run.
    xP = nc.dram_tensor("xP", [128, KC * S], BF, kind="ExternalInput").ap()
    wqP = nc.dram_tensor("wqP", [128, KC * QD], BF, kind="ExternalInput").ap()
    wkP = nc.dram_tensor("wkP", [128, KC * HD], BF, kind="ExternalInput").ap()
    wvP = nc.dram_tensor("wvP", [128, KC * HD], BF, kind="ExternalInput").ap()
    woP = nc.dram_tensor("woP", [128, GQ * H], BF, kind="ExternalInput").ap()
    cosT = nc.dram_tensor("cosT", [HD, S], BF, kind="ExternalInput").ap()
    sinS = nc.dram_tensor("sinS", [HD, S], BF, kind="ExternalInput").ap()
    bqkv = nc.dram_tensor("bqkv", [128, 6], F32, kind="ExternalInput").ap()
    onesd = nc.dram_tensor("onesd", [128, 128], BF, kind="ExternalInput").ap()
    outp = nc.dram_tensor("outp", [S, H], BF, kind="ExternalOutput").ap()

    with tile.TileContext(nc) as tc:
        with (
            tc.tile_pool(name="persist", bufs=1) as pp,
            tc.tile_pool(name="qfp", bufs=2) as pqf,
            tc.tile_pool(name="cd", bufs=1) as pd,
            tc.tile_pool(name="expp", bufs=1) as pe,
            tc.tile_pool(name="psum8", bufs=1, space="PSUM") as ps8,
        ):
            # persistent per-chunk K/V (split per t-chunk to keep dep ranges
            # disjoint between the producing chunk and attention readers)
            kf = [pp.tile([128, 512], BF, name=f"kf{t}") for t in range(TC)]
            v_sb = [pp.tile([128, 4, HD], BF, name=f"vsb{t}")
                    for t in range(TC)]
            ofl = pd.tile([128, GQ, S], BF)       # normalized attn outT

            # ---- constants ----
            bias_sb = pp.tile([128, 6], F32)
            nc.gpsimd.dma_start(bias_sb[:, :], bqkv)
            ident = pp.tile([128, 128], BF)
            make_identity(nc, ident[:, :])
            ones_mat = pp.tile([128, 128], BF)
            nc.gpsimd.dma_start(ones_mat[:, :], onesd)

            def jspan(qc, j):
                if j < 4 * qc:
                    q0, n = 512 * qc, 512
                else:
                    q0 = 128 * j
                    n = 512 * (qc + 1) - q0
                return q0, n, q0 - 512 * qc

            def attention(qc, qf_t, filler=None):
                """flash attention for q-chunk qc over k-tiles 0..4qc+3.

                filler(h) emits PE-dense side work (o_proj tiles of the
                previous chunk) interleaved per head, so the scheduler has
                matmuls to run while exp paces the score pipeline."""
                qs = slice(512 * qc, 512 * qc + 512)
                nj = 4 * qc + 4
                for h in range(GQ):
                    if filler is not None:
                        filler(h)
                    exs = pe.tile([128, 512], BF, tag="exs", bufs=3,
                                  name=f"exs_{h}_{qc}")
                    exts = []
                    for j in range(nj):
                        q0, n, off = jspan(qc, j)
                        ql = q0 - 512 * qc
                        ps = ps8.tile([128, 512], F32, tag=f"A{j % 4}",
                                      name=f"ps_{h}_{qc}_{j}")
                        nc.tensor.matmul(
                            ps[:, 0:n], kf[j // 4][:, 128 * (j % 4):
                                                   128 * (j % 4) + 128],
                            qf_t[:, h, ql:ql + n], start=True, stop=True)
                        ex = pe.tile([128, 512], BF, tag="E", bufs=24,
                                     name=f"ex_{h}_{qc}_{j}")
                        nc.scalar.activation(ex[:, 0:n], ps[:, 0:n], AF.Exp)
                        if j >= 4 * qc:
                            # zero the strictly-lower (q < k) triangle
                            nc.gpsimd.affine_select(
                                out=ex[:, 0:128], in_=ex[:, 0:128],
                                compare_op=mybir.AluOpType.is_ge, fill=0.0,
                                base=0, pattern=[[1, 128]],
                                channel_multiplier=-1)
                        if j == 0:
                            nc.vector.tensor_copy(exs[:, :], ex[:, :])
                        else:
                            nc.vector.tensor_add(exs[:, ql:ql + n],
                                                 exs[:, ql:ql + n],
                                                 ex[:, 0:n])
                        exts.append(ex)
                    p_sum = ps8.tile([128, 512], F32, tag="Bt", bufs=2,
                                     name=f"psum_{h}_{qc}")
                    nc.tensor.matmul(p_sum[:, :], ones_mat[:, :], exs[:, :],
                                     start=True, stop=True)
                    bc = pe.tile([128, 512], F32, tag="bc", bufs=2,
                                 name=f"bc_{h}_{qc}")
                    nc.vector.reciprocal_approx_fast(bc[:, :], p_sum[:, :])
                    p_o = ps8.tile([128, 512], F32, tag="Ct", bufs=2,
                                   name=f"po_{h}_{qc}")
                    for j in range(nj):
                        q0, n, off = jspan(qc, j)
                        nc.tensor.matmul(
                            p_o[:, off:off + n],
                            v_sb[j // 4][:, j % 4, :],
                            exts[j][:, 0:n], start=(j == 0), stop=(j == nj - 1))
                    nc.vector.tensor_mul(ofl[:, h, qs], p_o[:, :], bc[:, :])

            def oproj_tile(tt, pwo, wo_sb):
                """o_proj partial for one 128-token tile."""
                tsl = slice(128 * tt, 128 * tt + 128)
                tags = ["Bt", "Bt", "Ct", "Ct"]
                pfs = [ps8.tile([128, 512], F32, tag=tags[oc], bufs=2,
                                name=f"pf_{tt}_{oc}")
                       for oc in range(4)]
                for ic in range(GQ):
                    for oc in range(4):
                        osl = slice(512 * oc, 512 * oc + 512)
                        nc.tensor.matmul(
                            pfs[oc][:, :], ofl[:, ic, tsl],
                            wo_sb[:, ic, osl],
                            start=(ic == 0), stop=(ic == GQ - 1))
                fo = pwo.tile([128, 4, 512], BF, tag="fo", bufs=3,
                              name=f"fo_{tt}")
                for oc in range(4):
                    # split psum evictions between ACT and DVE
                    if oc < 2:
                        nc.scalar.copy(fo[:, oc, :], pfs[oc][:, :])
                    else:
                        nc.vector.tensor_copy(fo[:, oc, :], pfs[oc][:, :])
                nc.sync.dma_start(outp[tsl, :], fo[:, :, :])

            # ============ interleaved projections + attention =============
            qf_tiles = [None] * TC
            with (
                tc.tile_pool(name="projw", bufs=1) as pw,
                tc.tile_pool(name="rope", bufs=1) as pr,
                tc.tile_pool(name="wop", bufs=1) as pwo,
            ):
                wq_sb = pw.tile([128, KC, QD], BF)
                wk_sb = pw.tile([128, KC, HD], BF)
                wv_sb = pw.tile([128, KC, HD], BF)
                cos_sb = pw.tile([128, S], BF)
                sin_sb = pw.tile([128, S], BF)
                wo_sb = pwo.tile([128, GQ, H], BF)

                # All input DMAs on the SP HWDGE queue, in first-needed
                # order, sized so the ko=0 accumulation starts after ~1MB.
                x_sb = [None] * TC

                def ld_x(t, pieces):
                    xt = x_sb[t]
                    if xt is None:
                        xt = pw.tile([128, KC, 512], BF, tag="xc", bufs=2,
                                     name=f"x_sb{t}")
                        x_sb[t] = xt
                    for k0, k1 in pieces:
                        nc.sync.dma_start(
                            xt[:, k0:k1, :],
                            xP[:, 512 * (KC * t + k0):512 * (KC * t + k1)])

                def ld_wq(k0, k1):
                    nc.sync.dma_start(wq_sb[:, k0:k1, :],
                                      wqP[:, QD * k0:QD * k1])

                ld_x(0, [(0, 4)])
                ld_wq(0, 4)
                ld_x(0, [(4, 8)])
                ld_wq(4, 8)
                nc.sync.dma_start(wk_sb[:, :, :], wkP)
                nc.sync.dma_start(wv_sb[:, :, :], wvP)
                ld_x(0, [(8, 16)])
                ld_wq(8, 16)
                nc.sync.dma_start(cos_sb[:, :], cosT)
                nc.sync.dma_start(sin_sb[:, :], sinS)

                for t in range(TC):
                    ts = slice(512 * t, 512 * t + 512)
                    if t + 1 < TC:
                        ld_x(t + 1, [(0, 8), (8, 16)])
                    if t == 0:
                        nc.sync.dma_start(wo_sb[:, :, :], woP)

                    pq = [ps8.tile([128, 512], F32, tag=f"A{m}",
                                   name=f"pq{m}_{t}")
                          for m in range(GQ)]
                    pk = ps8.tile([128, 512], F32, tag="Bt", bufs=2,
                                  name=f"pk_{t}")
                    pv = ps8.tile([128, 512], F32, tag="Bt", bufs=2,
                                  name=f"pv_{t}")
                    for ko in range(KC):
                        st = (ko == 0)
                        sp = (ko == KC - 1)
                        xc = x_sb[t][:, ko, :]
                        for m in range(GQ):
                            nc.tensor.matmul(
                                pq[m][:, :],
                                wq_sb[:, ko, 128 * m:128 * m + 128],
                                xc, start=st, stop=sp)
                        nc.tensor.matmul(pk[:, :], wk_sb[:, ko, :],
                                         xc, start=st, stop=sp)
                        nc.tensor.matmul(pv[:, :], wv_sb[:, ko, :],
                                         xc, start=st, stop=sp)

                    # v: evict with bias, then transpose to natural layout
                    vT_t = pr.tile([128, 512], BF, tag="vT", bufs=2,
                                   name=f"vT_{t}")
                    nc.scalar.activation(vT_t[:, :], pv[:, :], AF.Identity,
                                         bias=bias_sb[:, 5:6])
                    for st4 in range(4):
                        ptr = ps8.tile([128, 128], BF, tag="Ct", bufs=2,
                                       name=f"ptr_{t}_{st4}")
                        nc.tensor.transpose(
                            ptr[:, :], vT_t[:, 128 * st4:128 * st4 + 128],
                            ident[:, :])
                        nc.scalar.copy(v_sb[t][:, st4, :], ptr[:, :])

                    # evict + bias; RoPE for q/k on DVE in [d, tok] layout
                    qf_t = pqf.tile([128, GQ, 512], BF, tag="qf",
                                    name=f"qf_{t}")
                    qf_tiles[t] = qf_t
                    for m in [GQ, 0, 1, 2, 3]:
                        raw = pr.tile([128, 512], BF, tag="raw", bufs=3,
                                      name=f"raw_{t}_{m}")
                        src_ps = pq[m][:, :] if m < GQ else pk[:, :]
                        bcol = m if m < GQ else 4
                        nc.scalar.activation(
                            raw[:, :], src_ps, AF.Identity,
                            bias=bias_sb[:, bcol:bcol + 1])
                        rot = pr.tile([128, 512], BF, tag="rot", bufs=2,
                                      name=f"rot_{t}_{m}")
                        nc.vector.tensor_copy(rot[0:64, :], raw[64:128, :])
                        nc.vector.tensor_copy(rot[64:128, :], raw[0:64, :])
                        t1 = pr.tile([128, 512], BF, tag="t1", bufs=2,
                                     name=f"t1_{t}_{m}")
                        nc.vector.tensor_mul(t1[:, :], rot[:, :],
                                             sin_sb[:, ts])
                        t2 = pr.tile([128, 512], BF, tag="t2", bufs=2,
                                     name=f"t2_{t}_{m}")
                        nc.vector.tensor_mul(t2[:, :], raw[:, :],
                                             cos_sb[:, ts])
                        dst = (qf_t[:, m, :] if m < GQ else kf[t][:, :])
                        nc.vector.tensor_add(dst, t1[:, :], t2[:, :])

                    # attention + o_proj for the PREVIOUS chunk run while
                    # this chunk's RoPE occupies the DVE; the o_proj tiles
                    # of chunk t-2 are interleaved per head as PE filler.
                    if t >= 1:
                        qc = t - 1
                        fill = (None if qc == 0 else
                                (lambda h, q=qc: oproj_tile(4 * (q - 1) + h,
                                                            pwo, wo_sb)))
                        attention(qc, qf_tiles[qc], filler=fill)
                qc = TC - 1
                attention(qc, qf_tiles[qc],
                          filler=lambda h: oproj_tile(4 * (qc - 1) + h,
                                                      pwo, wo_sb))
                for tt in range(4 * (TC - 1), 4 * TC):
                    oproj_tile(tt, pwo, wo_sb)


def _build():
    global _NC
    if _NC is None:
        nc = bacc.Bacc("TRN2", target_bir_lowering=False, debug=False,
                       num_devices=8)
        _emit(nc)
        nc.compile()
        _NC = nc
    return _NC


def _prep_inputs(x, wq, bq, wk, bk, wv, bv, wo, bo, cos, sin):
    """Host-side shard + layout prep. Core c = (g, b): g = c % 4, b = c // 4."""
    import ml_dtypes
    bf16 = ml_dtypes.bfloat16
    inv_sqrt_d = 1.0 / math.sqrt(HD)
    f32 = np.float32
    cosT = np.ascontiguousarray(cos.T.astype(bf16))
    sinSf = sin.T.astype(f32).copy()
    sinSf[0:HD // 2] *= -1.0
    sinS = np.ascontiguousarray(sinSf.astype(bf16))

    def pack(mT):
        """[n*128, m] -> [128, n*m]: row p = concat_n mT[n*128+p, :]."""
        n = mT.shape[0] // 128
        return np.ascontiguousarray(
            mT.reshape(n, 128, mT.shape[1]).transpose(1, 0, 2)
            .reshape(128, n * mT.shape[1]).astype(bf16))

    def pack_x(xT):
        """[H, S] -> [128, TC*KC*512]: [p, t, ko, c] = xT[ko*128+p, 512t+c]
        (token-chunk-major so each projection chunk is one contiguous run)."""
        return np.ascontiguousarray(
            xT.reshape(KC, 128, TC, 512).transpose(1, 2, 0, 3)
            .reshape(128, TC * KC * 512).astype(bf16))

    xPb = [pack_x(x[b].T.astype(f32)) for b in range(B)]

    in_maps = []
    for c in range(8):
        g, b = c % G, c // G
        wq_s = wq[QD * g:QD * (g + 1), :] * inv_sqrt_d
        bq_s = bq[QD * g:QD * (g + 1)] * inv_sqrt_d
        wk_s = wk[HD * g:HD * (g + 1), :]
        bk_s = bk[HD * g:HD * (g + 1)]
        wv_s = wv[HD * g:HD * (g + 1), :]
        bv_s = bv[HD * g:HD * (g + 1)]
        bias = np.zeros((128, 6), f32)
        bias[:, 0:4] = bq_s.reshape(GQ, HD).T
        bias[:, 4] = bk_s
        bias[:, 5] = bv_s
        in_maps.append({
            "xP": xPb[b],
            "wqP": pack(wq_s.T),
            "wkP": pack(wk_s.T),
            "wvP": pack(wv_s.T),
            "woP": pack(wo[:, QD * g:QD * (g + 1)].T),
            "cosT": cosT,
            "sinS": sinS,
            "bqkv": bias,
            "onesd": np.ones((128, 128), bf16),
        })
    return in_maps


def run(inputs, trace=False):
    """Returns (full_output, BassKernelResults)."""
    inputs = {k: np.asarray(v) for k, v in inputs.items()}
    nc = _build()
    in_maps = _prep_inputs(**inputs)
    res = run_bass_kernel_spmd(nc, in_maps, core_ids=list(range(8)),
                               trace=trace)
    bo = inputs["bo"].astype(np.float64)
    out = np.empty((B, S, H), np.float32)
    for b in range(B):
        acc = np.zeros((S, H), np.float64)
        for g in range(G):
            acc += res.results[G * b + g]["outp"].astype(np.float64)
        out[b] = (acc + bo).astype(np.float32)
    return out, res


def kernel(**inputs):
    return run(inputs, trace=False)[0]


# revision 20
# speedup vs baseline: 1.2149x; 1.2149x over previous
"""Trainium2 Bass kernel for GQA attention with RoPE (dense transformer).

Problem: B=2, S=2048, H=2048, 16 query heads / 4 KV heads, head_dim 128,
causal flash-style attention, fused QKV + o_proj.

Sharding (8 cores): (batch, head-group) grid. Core c handles batch c//4 and
head group c%4 (4 query heads + their shared KV head). o_proj is computed as
per-group partials reduced on host (tensor-parallel o_proj input split).

v2 vs baseline (336us):
  - bf16 activations/weights end-to-end (PE rate is identical to f32r, but
    DMA bytes and DVE element throughput both improve 2x; accuracy measured
    ~2e-3 rel, threshold 2e-2). PSUM accumulation stays fp32.
  - Softmax denominators: per (h,chunk) the exp tiles are accumulated over
    k-tiles on the DVE, then ONE ones-matmul computes the partition sum
    (was: one ones-matmul per k-tile; -144 PE matmuls).
  - Batched DMA: x loads as 1MB half-chunks, weights as whole tensors,
    outputs as one 0.5MB DMA per 128-token tile. Weights go on the ACT
    HWDGE queue, x/out on the SP queue.
  - o_proj is emitted interleaved (right after each q-chunk's attention),
    not as a tail phase, so its matmuls fill PE gaps left by exp pacing
    and the output DMA is spread across the kernel.

On-core layout: activations live as [feature, token] ("transposed") so the
feature contraction dims land on SBUF partitions for the PE array.
Causal masking: fully-masked k-tiles are skipped entirely; diagonal tiles
get a zero-fill triangle (affine_select on GpSimd) after exp.
"""
import math

import numpy as np

import concourse.bass as bass
import concourse.mybir as mybir
import concourse.tile as tile
from concourse import bacc
from concourse.bass_utils import run_bass_kernel_spmd
from concourse.masks import make_identity

B, S, H = 2, 2048, 2048
NH, KVH, HD = 16, 4, 128
G = 4                 # head groups (= KVH); grid = G x B = 8 cores
GQ = NH // KVH        # query heads per group
QD = GQ * HD          # per-core q dim (512)
KC = H // 128         # contraction chunks for projections (16)
TC = 4                # token chunks of 512
TT = S // 128         # 128-token tiles (16)

F32 = mybir.dt.float32
BF = mybir.dt.bfloat16
AF = mybir.ActivationFunctionType

_NC = None


def _emit(nc):
    # All big inputs are host-packed partition-major: row p is the full
    # contiguous per-partition payload, so every DMA is 128 descriptors of
    # >=4KB regardless of logical shape (HWDGE issue cost is ~5ns/descriptor).
    # x is packed token-chunk-major: [p, t, ko, c] so chunk t's whole
    # projection input is one contiguous 16KB-per-partition run.
    xP = nc.dram_tensor("xP", [128, KC * S], BF, kind="ExternalInput").ap()
    wqP = nc.dram_tensor("wqP", [128, KC * QD], BF, kind="ExternalInput").ap()
    wkP = nc.dram_tensor("wkP", [128, KC * HD], BF, kind="ExternalInput").ap()
    wvP = nc.dram_tensor("wvP", [128, KC * HD], BF, kind="ExternalInput").ap()
    woP = nc.dram_tensor("woP", [128, GQ * H], BF, kind="ExternalInput").ap()
    cosT = nc.dram_tensor("cosT", [HD, S], BF, kind="ExternalInput").ap()
    sinS = nc.dram_tensor("sinS", [HD, S], BF, kind="ExternalInput").ap()
    bqkv = nc.dram_tensor("bqkv", [128, 6], F32, kind="ExternalInput").ap()
    onesd = nc.dram_tensor("onesd", [128, 128], BF, kind="ExternalInput").ap()
    outp = nc.dram_tensor("outp", [S, H], BF, kind="ExternalOutput").ap()

    with tile.TileContext(nc) as tc:
        with (
            tc.tile_pool(name="persist", bufs=1) as pp,
            tc.tile_pool(name="qfp", bufs=2) as pqf,
            tc.tile_pool(name="cd", bufs=1) as pd,
            tc.tile_pool(name="expp", bufs=1) as pe,
            tc.tile_pool(name="psum8", bufs=1, space="PSUM") as ps8,
        ):
            # persistent per-chunk K/V (split per t-chunk to keep dep ranges
            # disjoint between the producing chunk and attention readers)
            kf = [pp.tile([128, 512], BF, name=f"kf{t}") for t in range(TC)]
            v_sb = [pp.tile([128, 4, HD], BF, name=f"vsb{t}")
                    for t in range(TC)]
            ofl = pd.tile([128, GQ, S], BF)       # normalized attn outT

            # ---- constants ----
            bias_sb = pp.tile([128, 6], F32)
            nc.gpsimd.dma_start(bias_sb[:, :], bqkv)
            ident = pp.tile([128, 128], BF)
            make_identity(nc, ident[:, :])
            ones_mat = pp.tile([128, 128], BF)
            nc.gpsimd.dma_start(ones_mat[:, :], onesd)

            def jspan(qc, j):
                if j < 4 * qc:
                    q0, n = 512 * qc, 512
                else:
                    q0 = 128 * j
                    n = 512 * (qc + 1) - q0
                return q0, n, q0 - 512 * qc

            def attention(qc, qf_t, filler=None):
                """flash attention for q-chunk qc over k-tiles 0..4qc+3.

                filler(h) emits PE-dense side work (o_proj tiles of the
                previous chunk) interleaved per head, so the scheduler has
                matmuls to run while exp paces the score pipeline."""
                qs = slice(512 * qc, 512 * qc + 512)
                nj = 4 * qc + 4
                state = {}

                def finish(h):
                    # softmax denominator + normalization for head h; emitted
                    # one head late so its ones-matmul never stalls PE on the
                    # DVE accumulation chain.
                    exs, p_o = state[h]
                    p_sum = ps8.tile([128, 512], F32, tag="Bt", bufs=2,
                                     name=f"psum_{h}_{qc}")
                    nc.tensor.matmul(p_sum[:, :], ones_mat[:, :], exs[:, :],
                                     start=True, stop=True)
                    bc = pe.tile([128, 512], F32, tag="bc", bufs=2,
                                 name=f"bc_{h}_{qc}")
                    nc.vector.reciprocal_approx_fast(bc[:, :], p_sum[:, :])
                    nc.vector.tensor_mul(ofl[:, h, qs], p_o[:, :], bc[:, :])

                for h in range(GQ):
                    if filler is not None:
                        filler(h)
                    exs = pe.tile([128, 512], BF, tag="exs", bufs=3,
                                  name=f"exs_{h}_{qc}")
                    exts = []
                    for j in range(nj):
                        q0, n, off = jspan(qc, j)
                        ql = q0 - 512 * qc
                        ps = ps8.tile([128, 512], F32, tag=f"A{j % 4}",
                                      name=f"ps_{h}_{qc}_{j}")
                        nc.tensor.matmul(
                            ps[:, 0:n], kf[j // 4][:, 128 * (j % 4):
                                                   128 * (j % 4) + 128],
                            qf_t[:, h, ql:ql + n], start=True, stop=True)
                        ex = pe.tile([128, 512], BF, tag="E", bufs=24,
                                     name=f"ex_{h}_{qc}_{j}")
                        nc.scalar.activation(ex[:, 0:n], ps[:, 0:n], AF.Exp)
                        if j >= 4 * qc:
                            # zero the strictly-lower (q < k) triangle
                            nc.gpsimd.affine_select(
                                out=ex[:, 0:128], in_=ex[:, 0:128],
                                compare_op=mybir.AluOpType.is_ge, fill=0.0,
                                base=0, pattern=[[1, 128]],
                                channel_multiplier=-1)
                        if j == 0:
                            nc.vector.tensor_copy(exs[:, :], ex[:, :])
                        else:
                            nc.vector.tensor_add(exs[:, ql:ql + n],
                                                 exs[:, ql:ql + n],
                                                 ex[:, 0:n])
                        exts.append(ex)
                    p_o = ps8.tile([128, 512], F32, tag="Ct", bufs=2,
                                   name=f"po_{h}_{qc}")
                    state[h] = (exs, p_o)
                    if h > 0:
                        finish(h - 1)
                    for j in range(nj):
                        q0, n, off = jspan(qc, j)
                        nc.tensor.matmul(
                            p_o[:, off:off + n],
                            v_sb[j // 4][:, j % 4, :],
                            exts[j][:, 0:n], start=(j == 0), stop=(j == nj - 1))
                finish(GQ - 1)

            def oproj_tile(tt, pwo, wo_sb):
                """o_proj partial for one 128-token tile.

                Two waves of 2 output-column groups on the Bt psum slots
                only, so the attention pipeline keeps both Ct slots for its
                held p_o accumulators."""
                tsl = slice(128 * tt, 128 * tt + 128)
                fo = pwo.tile([128, 4, 512], BF, tag="fo", bufs=3,
                              name=f"fo_{tt}")
                for w in range(2):
                    pfs = [ps8.tile([128, 512], F32, tag="Bt", bufs=2,
                                    name=f"pf_{tt}_{2 * w + i}")
                           for i in range(2)]
                    for ic in range(GQ):
                        for i in range(2):
                            oc = 2 * w + i
                            osl = slice(512 * oc, 512 * oc + 512)
                            nc.tensor.matmul(
                                pfs[i][:, :], ofl[:, ic, tsl],
                                wo_sb[:, ic, osl],
                                start=(ic == 0), stop=(ic == GQ - 1))
                    # split psum evictions between ACT and DVE
                    nc.scalar.copy(fo[:, 2 * w, :], pfs[0][:, :])
                    nc.vector.tensor_copy(fo[:, 2 * w + 1, :], pfs[1][:, :])
                nc.sync.dma_start(outp[tsl, :], fo[:, :, :])

            # ============ interleaved projections + attention =============
            qf_tiles = [None] * TC
            with (
                tc.tile_pool(name="projw", bufs=1) as pw,
                tc.tile_pool(name="rope", bufs=1) as pr,
                tc.tile_pool(name="wop", bufs=1) as pwo,
            ):
                wq_sb = pw.tile([128, KC, QD], BF)
                wk_sb = pw.tile([128, KC, HD], BF)
                wv_sb = pw.tile([128, KC, HD], BF)
                cos_sb = pw.tile([128, S], BF)
                sin_sb = pw.tile([128, S], BF)
                wo_sb = pwo.tile([128, GQ, H], BF)

                # All input DMAs on the SP HWDGE queue, in first-needed
                # order, sized so the ko=0 accumulation starts after ~1MB.
                x_sb = [None] * TC

                def ld_x(t, pieces):
                    xt = x_sb[t]
                    if xt is None:
                        xt = pw.tile([128, KC, 512], BF, tag="xc", bufs=2,
                                     name=f"x_sb{t}")
                        x_sb[t] = xt
                    for k0, k1 in pieces:
                        nc.sync.dma_start(
                            xt[:, k0:k1, :],
                            xP[:, 512 * (KC * t + k0):512 * (KC * t + k1)])

                def ld_wq(k0, k1):
                    nc.sync.dma_start(wq_sb[:, k0:k1, :],
                                      wqP[:, QD * k0:QD * k1])

                ld_x(0, [(0, 4)])
                ld_wq(0, 4)
                ld_x(0, [(4, 8)])
                ld_wq(4, 8)
                nc.sync.dma_start(wk_sb[:, :, :], wkP)
                nc.sync.dma_start(wv_sb[:, :, :], wvP)
                ld_x(0, [(8, 16)])
                ld_wq(8, 16)
                nc.sync.dma_start(cos_sb[:, :], cosT)
                nc.sync.dma_start(sin_sb[:, :], sinS)

                for t in range(TC):
                    ts = slice(512 * t, 512 * t + 512)
                    if t + 1 < TC:
                        ld_x(t + 1, [(0, 8), (8, 16)])
                    if t == 0:
                        nc.sync.dma_start(wo_sb[:, :, :], woP)

                    pq = [ps8.tile([128, 512], F32, tag=f"A{m}",
                                   name=f"pq{m}_{t}")
                          for m in range(GQ)]
                    pk = ps8.tile([128, 512], F32, tag="Bt", bufs=2,
                                  name=f"pk_{t}")
                    pv = ps8.tile([128, 512], F32, tag="Bt", bufs=2,
                                  name=f"pv_{t}")
                    for ko in range(KC):
                        st = (ko == 0)
                        sp = (ko == KC - 1)
                        xc = x_sb[t][:, ko, :]
                        for m in range(GQ):
                            nc.tensor.matmul(
                                pq[m][:, :],
                                wq_sb[:, ko, 128 * m:128 * m + 128],
                                xc, start=st, stop=sp)
                        nc.tensor.matmul(pk[:, :], wk_sb[:, ko, :],
                                         xc, start=st, stop=sp)
                        nc.tensor.matmul(pv[:, :], wv_sb[:, ko, :],
                                         xc, start=st, stop=sp)

                    # v: evict with bias, then transpose to natural layout
                    vT_t = pr.tile([128, 512], BF, tag="vT", bufs=2,
                                   name=f"vT_{t}")
                    nc.scalar.activation(vT_t[:, :], pv[:, :], AF.Identity,
                                         bias=bias_sb[:, 5:6])
                    for st4 in range(4):
                        ptr = ps8.tile([128, 128], BF, tag="Ct", bufs=2,
                                       name=f"ptr_{t}_{st4}")
                        nc.tensor.transpose(
                            ptr[:, :], vT_t[:, 128 * st4:128 * st4 + 128],
                            ident[:, :])
                        nc.scalar.copy(v_sb[t][:, st4, :], ptr[:, :])

                    # q/k psum evictions (+bias) on ACT first — this frees
                    # the A/Bt psum banks for the previous chunk's attention
                    raws = []
                    for m in [GQ, 0, 1, 2, 3]:
                        raw = pr.tile([128, 512], BF, tag="raw", bufs=6,
                                      name=f"raw_{t}_{m}")
                        src_ps = pq[m][:, :] if m < GQ else pk[:, :]
                        bcol = m if m < GQ else 4
                        nc.scalar.activation(
                            raw[:, :], src_ps, AF.Identity,
                            bias=bias_sb[:, bcol:bcol + 1])
                        raws.append((m, raw))

                    # attention + o_proj for the PREVIOUS chunk, emitted
                    # before this chunk's RoPE DVE chain so the attention's
                    # DVE work (exp sums, normalization) drains first; the
                    # o_proj tiles of chunk t-2 interleave per head as PE
                    # filler for the exp-paced score pipeline.
                    if t >= 1:
                        qc = t - 1
                        fill = (None if qc == 0 else
                                (lambda h, q=qc: oproj_tile(4 * (q - 1) + h,
                                                            pwo, wo_sb)))
                        attention(qc, qf_tiles[qc], filler=fill)

                    # RoPE for q/k on DVE in [d, tok] layout (needed only by
                    # THIS chunk's attention, one iteration later)
                    qf_t = pqf.tile([128, GQ, 512], BF, tag="qf",
                                    name=f"qf_{t}")
                    qf_tiles[t] = qf_t
                    for m, raw in raws:
                        rot = pr.tile([128, 512], BF, tag="rot", bufs=2,
                                      name=f"rot_{t}_{m}")
                        nc.vector.tensor_copy(rot[0:64, :], raw[64:128, :])
                        nc.vector.tensor_copy(rot[64:128, :], raw[0:64, :])
                        t1 = pr.tile([128, 512], BF, tag="t1", bufs=2,
                                     name=f"t1_{t}_{m}")
                        nc.vector.tensor_mul(t1[:, :], rot[:, :],
                                             sin_sb[:, ts])
                        t2 = pr.tile([128, 512], BF, tag="t2", bufs=2,
                                     name=f"t2_{t}_{m}")
                        nc.vector.tensor_mul(t2[:, :], raw[:, :],
                                             cos_sb[:, ts])
                        dst = (qf_t[:, m, :] if m < GQ else kf[t][:, :])
                        nc.vector.tensor_add(dst, t1[:, :], t2[:, :])
                qc = TC - 1
                attention(qc, qf_tiles[qc],
                          filler=lambda h: oproj_tile(4 * (qc - 1) + h,
                                                      pwo, wo_sb))
                for tt in range(4 * (TC - 1), 4 * TC):
                    oproj_tile(tt, pwo, wo_sb)


def _build():
    global _NC
    if _NC is None:
        nc = bacc.Bacc("TRN2", target_bir_lowering=False, debug=False,
                       num_devices=8)
        _emit(nc)
        nc.compile()
        _NC = nc
    return _NC


def _prep_inputs(x, wq, bq, wk, bk, wv, bv, wo, bo, cos, sin):
    """Host-side shard + layout prep. Core c = (g, b): g = c % 4, b = c // 4."""
    import ml_dtypes
    bf16 = ml_dtypes.bfloat16
    inv_sqrt_d = 1.0 / math.sqrt(HD)
    f32 = np.float32
    cosT = np.ascontiguousarray(cos.T.astype(bf16))
    sinSf = sin.T.astype(f32).copy()
    sinSf[0:HD // 2] *= -1.0
    sinS = np.ascontiguousarray(sinSf.astype(bf16))

    def pack(mT):
        """[n*128, m] -> [128, n*m]: row p = concat_n mT[n*128+p, :]."""
        n = mT.shape[0] // 128
        return np.ascontiguousarray(
            mT.reshape(n, 128, mT.shape[1]).transpose(1, 0, 2)
            .reshape(128, n * mT.shape[1]).astype(bf16))

    def pack_x(xT):
        """[H, S] -> [128, TC*KC*512]: [p, t, ko, c] = xT[ko*128+p, 512t+c]
        (token-chunk-major so each projection chunk is one contiguous run)."""
        return np.ascontiguousarray(
            xT.reshape(KC, 128, TC, 512).transpose(1, 2, 0, 3)
            .reshape(128, TC * KC * 512).astype(bf16))

    xPb = [pack_x(x[b].T.astype(f32)) for b in range(B)]

    in_maps = []
    for c in range(8):
        g, b = c % G, c // G
        wq_s = wq[QD * g:QD * (g + 1), :] * inv_sqrt_d
        bq_s = bq[QD * g:QD * (g + 1)] * inv_sqrt_d
        wk_s = wk[HD * g:HD * (g + 1), :]
        bk_s = bk[HD * g:HD * (g + 1)]
        wv_s = wv[HD * g:HD * (g + 1), :]
        bv_s = bv[HD * g:HD * (g + 1)]
        bias = np.zeros((128, 6), f32)
        bias[:, 0:4] = bq_s.reshape(GQ, HD).T
        bias[:, 4] = bk_s
        bias[:, 5] = bv_s
        in_maps.append({
            "xP": xPb[b],
            "wqP": pack(wq_s.T),
            "wkP": pack(wk_s.T),
            "wvP": pack(wv_s.T),
            "woP": pack(wo[:, QD * g:QD * (g + 1)].T),
            "cosT": cosT,
            "sinS": sinS,
            "bqkv": bias,
            "onesd": np.ones((128, 128), bf16),
        })
    return in_maps


def run(inputs, trace=False):
    """Returns (full_output, BassKernelResults)."""
    inputs = {k: np.asarray(v) for k, v in inputs.items()}
    nc = _build()
    in_maps = _prep_inputs(**inputs)
    res = run_bass_kernel_spmd(nc, in_maps, core_ids=list(range(8)),
                               trace=trace)
    bo = inputs["bo"].astype(np.float64)
    out = np.empty((B, S, H), np.float32)
    for b in range(B):
        acc = np.zeros((S, H), np.float64)
        for g in range(G):
            acc += res.results[G * b + g]["outp"].astype(np.float64)
        out[b] = (acc + bo).astype(np.float32)
    return out, res


def kernel(**inputs):
    return run(inputs, trace=False)[0]


# revision 26
# speedup vs baseline: 1.2545x; 1.0326x over previous
"""Trainium2 Bass kernel for GQA attention with RoPE (dense transformer).

Problem: B=2, S=2048, H=2048, 16 query heads / 4 KV heads, head_dim 128,
causal flash-style attention, fused QKV + o_proj.

Sharding (8 cores): (batch, head-group) grid. Core c handles batch c//4 and
head group c%4 (4 query heads + their shared KV head). o_proj is computed as
per-group partials reduced on host (tensor-parallel o_proj input split).

v2 vs baseline (336us):
  - bf16 activations/weights end-to-end (PE rate is identical to f32r, but
    DMA bytes and DVE element throughput both improve 2x; accuracy measured
    ~2e-3 rel, threshold 2e-2). PSUM accumulation stays fp32.
  - Softmax denominators: per (h,chunk) the exp tiles are accumulated over
    k-tiles on the DVE, then ONE ones-matmul computes the partition sum
    (was: one ones-matmul per k-tile; -144 PE matmuls).
  - Batched DMA: x loads as 1MB half-chunks, weights as whole tensors,
    outputs as one 0.5MB DMA per 128-token tile. Weights go on the ACT
    HWDGE queue, x/out on the SP queue.
  - o_proj is emitted interleaved (right after each q-chunk's attention),
    not as a tail phase, so its matmuls fill PE gaps left by exp pacing
    and the output DMA is spread across the kernel.

On-core layout: activations live as [feature, token] ("transposed") so the
feature contraction dims land on SBUF partitions for the PE array.
Causal masking: fully-masked k-tiles are skipped entirely; diagonal tiles
get a zero-fill triangle (affine_select on GpSimd) after exp.
"""
import math

import numpy as np

import concourse.bass as bass
import concourse.mybir as mybir
import concourse.tile as tile
from concourse import bacc
from concourse.bass_utils import run_bass_kernel_spmd
from concourse.masks import make_identity

B, S, H = 2, 2048, 2048
NH, KVH, HD = 16, 4, 128
G = 4                 # head groups (= KVH); grid = G x B = 8 cores
GQ = NH // KVH        # query heads per group
QD = GQ * HD          # per-core q dim (512)
KC = H // 128         # contraction chunks for projections (16)
TC = 4                # token chunks of 512
TT = S // 128         # 128-token tiles (16)

F32 = mybir.dt.float32
BF = mybir.dt.bfloat16
AF = mybir.ActivationFunctionType

_NC = None


def _emit(nc):
    # All big inputs are host-packed partition-major: row p is the full
    # contiguous per-partition payload, so every DMA is 128 descriptors of
    # >=4KB regardless of logical shape (HWDGE issue cost is ~5ns/descriptor).
    # x is packed token-chunk-major: [p, t, ko, c] so chunk t's whole
    # projection input is one contiguous 16KB-per-partition run.
    xP = nc.dram_tensor("xP", [128, KC * S], BF, kind="ExternalInput").ap()
    wqP = nc.dram_tensor("wqP", [128, KC * QD], BF, kind="ExternalInput").ap()
    wkP = nc.dram_tensor("wkP", [128, KC * HD], BF, kind="ExternalInput").ap()
    wvP = nc.dram_tensor("wvP", [128, KC * HD], BF, kind="ExternalInput").ap()
    woP = nc.dram_tensor("woP", [128, GQ * H], BF, kind="ExternalInput").ap()
    cosT = nc.dram_tensor("cosT", [HD, S], BF, kind="ExternalInput").ap()
    sinS = nc.dram_tensor("sinS", [HD, S], BF, kind="ExternalInput").ap()
    bqkv = nc.dram_tensor("bqkv", [128, 6], F32, kind="ExternalInput").ap()
    onesd = nc.dram_tensor("onesd", [128, 128], BF, kind="ExternalInput").ap()
    outp = nc.dram_tensor("outp", [S, H], BF, kind="ExternalOutput").ap()

    with tile.TileContext(nc) as tc:
        with (
            tc.tile_pool(name="persist", bufs=1) as pp,
            tc.tile_pool(name="qfp", bufs=2) as pqf,
            tc.tile_pool(name="cd", bufs=1) as pd,
            tc.tile_pool(name="expp", bufs=1) as pe,
            tc.tile_pool(name="psum8", bufs=1, space="PSUM") as ps8,
        ):
            # persistent per-chunk K/V (split per t-chunk to keep dep ranges
            # disjoint between the producing chunk and attention readers)
            kf = [pp.tile([128, 512], BF, name=f"kf{t}") for t in range(TC)]
            v_sb = [pp.tile([128, 4, HD], BF, name=f"vsb{t}")
                    for t in range(TC)]
            ofl = pd.tile([128, GQ, S], BF)       # normalized attn outT

            # ---- constants ----
            bias_sb = pp.tile([128, 6], F32)
            nc.gpsimd.dma_start(bias_sb[:, :], bqkv)
            ident = pp.tile([128, 128], BF)
            make_identity(nc, ident[:, :])
            ones_mat = pp.tile([128, 128], BF)
            nc.gpsimd.dma_start(ones_mat[:, :], onesd)

            def jspan(qc, j):
                if j < 4 * qc:
                    q0, n = 512 * qc, 512
                else:
                    q0 = 128 * j
                    n = 512 * (qc + 1) - q0
                return q0, n, q0 - 512 * qc

            def attention(qc, qf_t, filler=None):
                """flash attention for q-chunk qc over k-tiles 0..4qc+3.

                filler(h) emits PE-dense side work (o_proj tiles of the
                previous chunk) interleaved per head, so the scheduler has
                matmuls to run while exp paces the score pipeline."""
                qs = slice(512 * qc, 512 * qc + 512)
                nj = 4 * qc + 4
                state = {}

                def finish(h):
                    # softmax denominator + normalization for head h; emitted
                    # one head late so its ones-matmul never stalls PE on the
                    # DVE accumulation chain.
                    exs, p_o = state[h]
                    p_sum = ps8.tile([128, 512], F32, tag="Bt", bufs=2,
                                     name=f"psum_{h}_{qc}")
                    nc.tensor.matmul(p_sum[:, :], ones_mat[:, :], exs[:, :],
                                     start=True, stop=True)
                    bc = pe.tile([128, 512], F32, tag="bc", bufs=2,
                                 name=f"bc_{h}_{qc}")
                    nc.vector.reciprocal_approx_fast(bc[:, :], p_sum[:, :])
                    nc.vector.tensor_mul(ofl[:, h, qs], p_o[:, :], bc[:, :])

                for h in range(GQ):
                    if filler is not None:
                        filler(h)
                    exs = pe.tile([128, 512], BF, tag="exs", bufs=3,
                                  name=f"exs_{h}_{qc}")
                    exts = []
                    for j in range(nj):
                        q0, n, off = jspan(qc, j)
                        ql = q0 - 512 * qc
                        ps = ps8.tile([128, 512], F32, tag=f"A{j % 4}",
                                      name=f"ps_{h}_{qc}_{j}")
                        nc.tensor.matmul(
                            ps[:, 0:n], kf[j // 4][:, 128 * (j % 4):
                                                   128 * (j % 4) + 128],
                            qf_t[:, h, ql:ql + n], start=True, stop=True)
                        ex = pe.tile([128, 512], BF, tag="E", bufs=24,
                                     name=f"ex_{h}_{qc}_{j}")
                        nc.scalar.activation(ex[:, 0:n], ps[:, 0:n], AF.Exp)
                        if j >= 4 * qc:
                            # zero the strictly-lower (q < k) triangle
                            nc.gpsimd.affine_select(
                                out=ex[:, 0:128], in_=ex[:, 0:128],
                                compare_op=mybir.AluOpType.is_ge, fill=0.0,
                                base=0, pattern=[[1, 128]],
                                channel_multiplier=-1)
                        if j == 0:
                            nc.vector.tensor_copy(exs[:, :], ex[:, :])
                        else:
                            nc.vector.tensor_add(exs[:, ql:ql + n],
                                                 exs[:, ql:ql + n],
                                                 ex[:, 0:n])
                        exts.append(ex)
                    p_o = ps8.tile([128, 512], F32, tag="Ct", bufs=2,
                                   name=f"po_{h}_{qc}")
                    state[h] = (exs, p_o)
                    for j in range(nj):
                        q0, n, off = jspan(qc, j)
                        nc.tensor.matmul(
                            p_o[:, off:off + n],
                            v_sb[j // 4][:, j % 4, :],
                            exts[j][:, 0:n], start=(j == 0), stop=(j == nj - 1))
                    if h > 0:
                        finish(h - 1)
                finish(GQ - 1)

            def oproj_tile(tt, pwo, wo_sb, tail=False):
                """o_proj partial for one 128-token tile.

                Interleaved (filler) mode runs two waves of 2 output-column
                groups on the Bt psum slots only, so the attention pipeline
                keeps both Ct slots for its held p_o accumulators. Tail mode
                (after the last attention chunk) runs one 4-slot wave using
                the then-idle score banks."""
                tsl = slice(128 * tt, 128 * tt + 128)
                fo = pwo.tile([128, 4, 512], BF, tag="fo", bufs=3,
                              name=f"fo_{tt}")
                waves = ([("Bt", "Bt", "A0", "A1")] if tail
                         else [("Bt", "Bt"), ("Bt", "Bt")])
                oc = 0
                for w, tags in enumerate(waves):
                    pfs = [ps8.tile([128, 512], F32, tag=tg,
                                    bufs=(2 if tg in ("Bt", "Ct") else 1),
                                    name=f"pf_{tt}_{oc + i}")
                           for i, tg in enumerate(tags)]
                    for ic in range(GQ):
                        for i in range(len(tags)):
                            osl = slice(512 * (oc + i), 512 * (oc + i) + 512)
                            nc.tensor.matmul(
                                pfs[i][:, :], ofl[:, ic, tsl],
                                wo_sb[:, ic, osl],
                                start=(ic == 0), stop=(ic == GQ - 1))
                    # split psum evictions between ACT and DVE
                    for i in range(len(tags)):
                        if i % 2 == 0:
                            nc.scalar.copy(fo[:, oc + i, :], pfs[i][:, :])
                        else:
                            nc.vector.tensor_copy(fo[:, oc + i, :],
                                                  pfs[i][:, :])
                    oc += len(tags)
                nc.sync.dma_start(outp[tsl, :], fo[:, :, :])

            # ============ interleaved projections + attention =============
            qf_tiles = [None] * TC
            with (
                tc.tile_pool(name="projw", bufs=1) as pw,
                tc.tile_pool(name="rope", bufs=1) as pr,
                tc.tile_pool(name="wop", bufs=1) as pwo,
            ):
                wq_sb = pw.tile([128, KC, QD], BF)
                wk_sb = pw.tile([128, KC, HD], BF)
                wv_sb = pw.tile([128, KC, HD], BF)
                cos_sb = pw.tile([128, S], BF)
                sin_sb = pw.tile([128, S], BF)
                wo_sb = pwo.tile([128, GQ, H], BF)

                # All input DMAs on the SP HWDGE queue, in first-needed
                # order, sized so the ko=0 accumulation starts after ~1MB.
                x_sb = [None] * TC

                def ld_x(t, pieces):
                    xt = x_sb[t]
                    if xt is None:
                        xt = pw.tile([128, KC, 512], BF, tag="xc", bufs=2,
                                     name=f"x_sb{t}")
                        x_sb[t] = xt
                    for k0, k1 in pieces:
                        nc.sync.dma_start(
                            xt[:, k0:k1, :],
                            xP[:, 512 * (KC * t + k0):512 * (KC * t + k1)])

                def ld_wq(k0, k1):
                    nc.sync.dma_start(wq_sb[:, k0:k1, :],
                                      wqP[:, QD * k0:QD * k1])

                ld_x(0, [(0, 2)])
                ld_wq(0, 2)
                ld_x(0, [(2, 4)])
                ld_wq(2, 4)
                nc.sync.dma_start(wk_sb[:, :, :], wkP)
                nc.sync.dma_start(wv_sb[:, :, :], wvP)
                ld_x(0, [(4, 8)])
                ld_wq(4, 8)
                ld_x(0, [(8, 16)])
                ld_wq(8, 16)
                nc.sync.dma_start(cos_sb[:, :], cosT)
                nc.sync.dma_start(sin_sb[:, :], sinS)

                for t in range(TC):
                    ts = slice(512 * t, 512 * t + 512)
                    if t + 1 < TC:
                        ld_x(t + 1, [(0, 8), (8, 16)])
                    if t == 0:
                        nc.sync.dma_start(wo_sb[:, :, :], woP)

                    pq = [ps8.tile([128, 512], F32, tag=f"A{m}",
                                   name=f"pq{m}_{t}")
                          for m in range(GQ)]
                    pk = ps8.tile([128, 512], F32, tag="Bt", bufs=2,
                                  name=f"pk_{t}")
                    pv = ps8.tile([128, 512], F32, tag="Bt", bufs=2,
                                  name=f"pv_{t}")
                    for ko in range(KC):
                        st = (ko == 0)
                        sp = (ko == KC - 1)
                        xc = x_sb[t][:, ko, :]
                        for m in range(GQ):
                            nc.tensor.matmul(
                                pq[m][:, :],
                                wq_sb[:, ko, 128 * m:128 * m + 128],
                                xc, start=st, stop=sp)
                        nc.tensor.matmul(pk[:, :], wk_sb[:, ko, :],
                                         xc, start=st, stop=sp)
                        nc.tensor.matmul(pv[:, :], wv_sb[:, ko, :],
                                         xc, start=st, stop=sp)

                    # q/k psum evictions (+bias) on ACT first — q heads
                    # before k so the A psum banks free up one by one for
                    # the previous chunk's attention scores
                    raws = []
                    for m in [0, 1, 2, 3, GQ]:
                        raw = pr.tile([128, 512], BF, tag="raw", bufs=6,
                                      name=f"raw_{t}_{m}")
                        src_ps = pq[m][:, :] if m < GQ else pk[:, :]
                        bcol = m if m < GQ else 4
                        nc.scalar.activation(
                            raw[:, :], src_ps, AF.Identity,
                            bias=bias_sb[:, bcol:bcol + 1])
                        raws.append((m, raw))

                    # v: evict with bias, then transpose to natural layout
                    # (only needed by THIS chunk's attention, next iteration)
                    vT_t = pr.tile([128, 512], BF, tag="vT", bufs=2,
                                   name=f"vT_{t}")
                    nc.scalar.activation(vT_t[:, :], pv[:, :], AF.Identity,
                                         bias=bias_sb[:, 5:6])
                    for st4 in range(4):
                        ptr = ps8.tile([128, 128], BF, tag="Ct", bufs=2,
                                       name=f"ptr_{t}_{st4}")
                        nc.tensor.transpose(
                            ptr[:, :], vT_t[:, 128 * st4:128 * st4 + 128],
                            ident[:, :])
                        nc.scalar.copy(v_sb[t][:, st4, :], ptr[:, :])

                    # attention + o_proj for the PREVIOUS chunk, emitted
                    # before this chunk's RoPE DVE chain so the attention's
                    # DVE work (exp sums, normalization) drains first; the
                    # o_proj tiles of chunk t-2 interleave per head as PE
                    # filler for the exp-paced score pipeline.
                    if t >= 1:
                        qc = t - 1
                        fill = (None if qc == 0 else
                                (lambda h, q=qc: oproj_tile(4 * (q - 1) + h,
                                                            pwo, wo_sb)))
                        attention(qc, qf_tiles[qc], filler=fill)

                    # RoPE for q/k on DVE in [d, tok] layout (needed only by
                    # THIS chunk's attention, one iteration later)
                    qf_t = pqf.tile([128, GQ, 512], BF, tag="qf",
                                    name=f"qf_{t}")
                    qf_tiles[t] = qf_t
                    for m, raw in raws:
                        rot = pr.tile([128, 512], BF, tag="rot", bufs=2,
                                      name=f"rot_{t}_{m}")
                        nc.vector.tensor_copy(rot[0:64, :], raw[64:128, :])
                        nc.vector.tensor_copy(rot[64:128, :], raw[0:64, :])
                        t1 = pr.tile([128, 512], BF, tag="t1", bufs=2,
                                     name=f"t1_{t}_{m}")
                        nc.vector.tensor_mul(t1[:, :], rot[:, :],
                                             sin_sb[:, ts])
                        t2 = pr.tile([128, 512], BF, tag="t2", bufs=2,
                                     name=f"t2_{t}_{m}")
                        nc.vector.tensor_mul(t2[:, :], raw[:, :],
                                             cos_sb[:, ts])
                        dst = (qf_t[:, m, :] if m < GQ else kf[t][:, :])
                        nc.vector.tensor_add(dst, t1[:, :], t2[:, :])
                qc = TC - 1
                attention(qc, qf_tiles[qc],
                          filler=lambda h: oproj_tile(4 * (qc - 1) + h,
                                                      pwo, wo_sb))
                for tt in range(4 * (TC - 1), 4 * TC):
                    oproj_tile(tt, pwo, wo_sb, tail=True)


def _build():
    global _NC
    if _NC is None:
        nc = bacc.Bacc("TRN2", target_bir_lowering=False, debug=False,
                       num_devices=8)
        _emit(nc)
        nc.compile()
        _NC = nc
    return _NC


def _prep_inputs(x, wq, bq, wk, bk, wv, bv, wo, bo, cos, sin):
    """Host-side shard + layout prep. Core c = (g, b): g = c % 4, b = c // 4."""
    import ml_dtypes
    bf16 = ml_dtypes.bfloat16
    inv_sqrt_d = 1.0 / math.sqrt(HD)
    f32 = np.float32
    cosT = np.ascontiguousarray(cos.T.astype(bf16))
    sinSf = sin.T.astype(f32).copy()
    sinSf[0:HD // 2] *= -1.0
    sinS = np.ascontiguousarray(sinSf.astype(bf16))

    def pack(mT):
        """[n*128, m] -> [128, n*m]: row p = concat_n mT[n*128+p, :]."""
        n = mT.shape[0] // 128
        return np.ascontiguousarray(
            mT.reshape(n, 128, mT.shape[1]).transpose(1, 0, 2)
            .reshape(128, n * mT.shape[1]).astype(bf16))

    def pack_x(xT):
        """[H, S] -> [128, TC*KC*512]: [p, t, ko, c] = xT[ko*128+p, 512t+c]
        (token-chunk-major so each projection chunk is one contiguous run)."""
        return np.ascontiguousarray(
            xT.reshape(KC, 128, TC, 512).transpose(1, 2, 0, 3)
            .reshape(128, TC * KC * 512).astype(bf16))

    xPb = [pack_x(x[b].T.astype(f32)) for b in range(B)]

    in_maps = []
    for c in range(8):
        g, b = c % G, c // G
        wq_s = wq[QD * g:QD * (g + 1), :] * inv_sqrt_d
        bq_s = bq[QD * g:QD * (g + 1)] * inv_sqrt_d
        wk_s = wk[HD * g:HD * (g + 1), :]
        bk_s = bk[HD * g:HD * (g + 1)]
        wv_s = wv[HD * g:HD * (g + 1), :]
        bv_s = bv[HD * g:HD * (g + 1)]
        bias = np.zeros((128, 6), f32)
        bias[:, 0:4] = bq_s.reshape(GQ, HD).T
        bias[:, 4] = bk_s
        bias[:, 5] = bv_s
        in_maps.append({
            "xP": xPb[b],
            "wqP": pack(wq_s.T),
            "wkP": pack(wk_s.T),
            "wvP": pack(wv_s.T),
            "woP": pack(wo[:, QD * g:QD * (g + 1)].T),
            "cosT": cosT,
            "sinS": sinS,
            "bqkv": bias,
            "onesd": np.ones((128, 128), bf16),
        })
    return in_maps


def run(inputs, trace=False):
    """Returns (full_output, BassKernelResults)."""
    inputs = {k: np.asarray(v) for k, v in inputs.items()}
    nc = _build()
    in_maps = _prep_inputs(**inputs)
    res = run_bass_kernel_spmd(nc, in_maps, core_ids=list(range(8)),
                               trace=trace)
    bo = inputs["bo"].astype(np.float64)
    out = np.empty((B, S, H), np.float32)
    for b in range(B):
        acc = np.zeros((S, H), np.float64)
        for g in range(G):
            acc += res.results[G * b + g]["outp"].astype(np.float64)
        out[b] = (acc + bo).astype(np.float32)
    return out, res


def kernel(**inputs):
    return run(inputs, trace=False)[0]


# revision 32
# speedup vs baseline: 1.2740x; 1.0155x over previous
"""Trainium2 Bass kernel for GQA attention with RoPE (dense transformer).

Problem: B=2, S=2048, H=2048, 16 query heads / 4 KV heads, head_dim 128,
causal flash-style attention, fused QKV + o_proj.

Sharding (8 cores): (batch, head-group) grid. Core c handles batch c//4 and
head group c%4 (4 query heads + their shared KV head). o_proj is computed as
per-group partials reduced on host (tensor-parallel o_proj input split).

v2 vs baseline (336us):
  - bf16 activations/weights end-to-end (PE rate is identical to f32r, but
    DMA bytes and DVE element throughput both improve 2x; accuracy measured
    ~2e-3 rel, threshold 2e-2). PSUM accumulation stays fp32.
  - Softmax denominators: per (h,chunk) the exp tiles are accumulated over
    k-tiles on the DVE, then ONE ones-matmul computes the partition sum
    (was: one ones-matmul per k-tile; -144 PE matmuls).
  - Batched DMA: x loads as 1MB half-chunks, weights as whole tensors,
    outputs as one 0.5MB DMA per 128-token tile. Weights go on the ACT
    HWDGE queue, x/out on the SP queue.
  - o_proj is emitted interleaved (right after each q-chunk's attention),
    not as a tail phase, so its matmuls fill PE gaps left by exp pacing
    and the output DMA is spread across the kernel.

On-core layout: activations live as [feature, token] ("transposed") so the
feature contraction dims land on SBUF partitions for the PE array.
Causal masking: fully-masked k-tiles are skipped entirely; diagonal tiles
get a zero-fill triangle (affine_select on GpSimd) after exp.
"""
import math

import numpy as np

import concourse.bass as bass
import concourse.mybir as mybir
import concourse.tile as tile
from concourse import bacc
from concourse.bass_utils import run_bass_kernel_spmd
from concourse.masks import make_identity

B, S, H = 2, 2048, 2048
NH, KVH, HD = 16, 4, 128
G = 4                 # head groups (= KVH); grid = G x B = 8 cores
GQ = NH // KVH        # query heads per group
QD = GQ * HD          # per-core q dim (512)
KC = H // 128         # contraction chunks for projections (16)
TC = 4                # token chunks of 512
TT = S // 128         # 128-token tiles (16)

F32 = mybir.dt.float32
BF = mybir.dt.bfloat16
AF = mybir.ActivationFunctionType

_NC = None


def _emit(nc):
    # All big inputs are host-packed partition-major: row p is the full
    # contiguous per-partition payload, so every DMA is 128 descriptors of
    # >=4KB regardless of logical shape (HWDGE issue cost is ~5ns/descriptor).
    # x is packed token-chunk-major: [p, t, ko, c] so chunk t's whole
    # projection input is one contiguous 16KB-per-partition run.
    xP = nc.dram_tensor("xP", [128, KC * S], BF, kind="ExternalInput").ap()
    wqP = nc.dram_tensor("wqP", [128, KC * QD], BF, kind="ExternalInput").ap()
    wkP = nc.dram_tensor("wkP", [128, KC * HD], BF, kind="ExternalInput").ap()
    wvP = nc.dram_tensor("wvP", [128, KC * HD], BF, kind="ExternalInput").ap()
    woP = nc.dram_tensor("woP", [128, GQ * H], BF, kind="ExternalInput").ap()
    cosT = nc.dram_tensor("cosT", [HD, S], BF, kind="ExternalInput").ap()
    sinS = nc.dram_tensor("sinS", [HD, S], BF, kind="ExternalInput").ap()
    bqkv = nc.dram_tensor("bqkv", [128, 6], F32, kind="ExternalInput").ap()
    onesd = nc.dram_tensor("onesd", [128, 128], BF, kind="ExternalInput").ap()
    outp = nc.dram_tensor("outp", [S, H], BF, kind="ExternalOutput").ap()

    with tile.TileContext(nc) as tc:
        with (
            tc.tile_pool(name="persist", bufs=1) as pp,
            tc.tile_pool(name="qfp", bufs=2) as pqf,
            tc.tile_pool(name="cd", bufs=1) as pd,
            tc.tile_pool(name="expp", bufs=1) as pe,
            tc.tile_pool(name="psum8", bufs=1, space="PSUM") as ps8,
        ):
            # persistent per-chunk K/V (split per t-chunk to keep dep ranges
            # disjoint between the producing chunk and attention readers)
            kf = [pp.tile([128, 512], BF, name=f"kf{t}") for t in range(TC)]
            v_sb = [pp.tile([128, 4, HD], BF, name=f"vsb{t}")
                    for t in range(TC)]
            ofl = pd.tile([128, GQ, S], BF)       # normalized attn outT

            # ---- constants ----
            bias_sb = pp.tile([128, 6], F32)
            nc.gpsimd.dma_start(bias_sb[:, :], bqkv)
            ident = pp.tile([128, 128], BF)
            make_identity(nc, ident[:, :])
            ones_mat = pp.tile([128, 128], BF)
            nc.gpsimd.dma_start(ones_mat[:, :], onesd)

            def jspan(qc, j):
                if j < 4 * qc:
                    q0, n = 512 * qc, 512
                else:
                    q0 = 128 * j
                    n = 512 * (qc + 1) - q0
                return q0, n, q0 - 512 * qc

            def attention(qc, qf_t, filler=None):
                """flash attention for q-chunk qc over k-tiles 0..4qc+3.

                filler(h) emits PE-dense side work (o_proj tiles of the
                previous chunk) interleaved per head, so the scheduler has
                matmuls to run while exp paces the score pipeline."""
                qs = slice(512 * qc, 512 * qc + 512)
                nj = 4 * qc + 4
                state = {}

                def finish(h):
                    # softmax denominator + normalization for head h; emitted
                    # one head late so its ones-matmul never stalls PE on the
                    # DVE accumulation chain.
                    exs, p_o = state[h]
                    p_sum = ps8.tile([128, 512], F32, tag="Bt", bufs=2,
                                     name=f"psum_{h}_{qc}")
                    nc.tensor.matmul(p_sum[:, :], ones_mat[:, :], exs[:, :],
                                     start=True, stop=True)
                    bc = pe.tile([128, 512], F32, tag="bc", bufs=2,
                                 name=f"bc_{h}_{qc}")
                    nc.vector.reciprocal_approx_fast(bc[:, :], p_sum[:, :])
                    nc.vector.tensor_mul(ofl[:, h, qs], p_o[:, :], bc[:, :])

                for h in range(GQ):
                    if filler is not None:
                        filler(h)
                    exs = pe.tile([128, 512], BF, tag="exs", bufs=3,
                                  name=f"exs_{h}_{qc}")
                    exts = []
                    for j in range(nj):
                        q0, n, off = jspan(qc, j)
                        ql = q0 - 512 * qc
                        ps = ps8.tile([128, 512], F32, tag=f"A{j % 4}",
                                      name=f"ps_{h}_{qc}_{j}")
                        nc.tensor.matmul(
                            ps[:, 0:n], kf[j // 4][:, 128 * (j % 4):
                                                   128 * (j % 4) + 128],
                            qf_t[:, h, ql:ql + n], start=True, stop=True)
                        ex = pe.tile([128, 512], BF, tag="E", bufs=28,
                                     name=f"ex_{h}_{qc}_{j}")
                        nc.scalar.activation(ex[:, 0:n], ps[:, 0:n], AF.Exp)
                        if j >= 4 * qc:
                            # zero the strictly-lower (q < k) triangle
                            nc.gpsimd.affine_select(
                                out=ex[:, 0:128], in_=ex[:, 0:128],
                                compare_op=mybir.AluOpType.is_ge, fill=0.0,
                                base=0, pattern=[[1, 128]],
                                channel_multiplier=-1)
                        if j == 0:
                            nc.vector.tensor_copy(exs[:, :], ex[:, :])
                        else:
                            nc.vector.tensor_add(exs[:, ql:ql + n],
                                                 exs[:, ql:ql + n],
                                                 ex[:, 0:n])
                        exts.append(ex)
                    p_o = ps8.tile([128, 512], F32, tag="Ct", bufs=2,
                                   name=f"po_{h}_{qc}")
                    state[h] = (exs, p_o)
                    for j in range(nj):
                        q0, n, off = jspan(qc, j)
                        nc.tensor.matmul(
                            p_o[:, off:off + n],
                            v_sb[j // 4][:, j % 4, :],
                            exts[j][:, 0:n], start=(j == 0), stop=(j == nj - 1))
                    if h > 0:
                        finish(h - 1)
                finish(GQ - 1)

            def oproj_tile(tt, pwo, wo_sb, tail=False, last=False):
                """o_proj partial for one 128-token tile.

                Interleaved (filler) mode runs two waves of 2 output-column
                groups on the Bt psum slots only, so the attention pipeline
                keeps both Ct slots for its held p_o accumulators. Tail mode
                (after the last attention chunk) runs one 4-slot wave using
                the then-idle score banks."""
                tsl = slice(128 * tt, 128 * tt + 128)
                fo = pwo.tile([128, 4, 512], BF, tag="fo", bufs=3,
                              name=f"fo_{tt}")
                waves = ([("Bt", "Bt", "A0", "A1")] if tail
                         else [("Bt", "Bt"), ("Bt", "Bt")])
                oc = 0
                for w, tags in enumerate(waves):
                    pfs = [ps8.tile([128, 512], F32, tag=tg,
                                    bufs=(2 if tg in ("Bt", "Ct") else 1),
                                    name=f"pf_{tt}_{oc + i}")
                           for i, tg in enumerate(tags)]
                    for ic in range(GQ):
                        for i in range(len(tags)):
                            osl = slice(512 * (oc + i), 512 * (oc + i) + 512)
                            nc.tensor.matmul(
                                pfs[i][:, :], ofl[:, ic, tsl],
                                wo_sb[:, ic, osl],
                                start=(ic == 0), stop=(ic == GQ - 1))
                    # split psum evictions between ACT and DVE
                    for i in range(len(tags)):
                        if i % 2 == 0:
                            nc.scalar.copy(fo[:, oc + i, :], pfs[i][:, :])
                        else:
                            nc.vector.tensor_copy(fo[:, oc + i, :],
                                                  pfs[i][:, :])
                        if last:
                            # final tile: ship each quarter as it lands so
                            # the kernel-exit barrier isn't gated on one
                            # full-tile DMA at the very end
                            o = oc + i
                            nc.sync.dma_start(
                                outp[tsl, 512 * o:512 * o + 512],
                                fo[:, o, :])
                    oc += len(tags)
                if not last:
                    nc.sync.dma_start(outp[tsl, :], fo[:, :, :])

            # ============ interleaved projections + attention =============
            qf_tiles = [None] * TC
            with (
                tc.tile_pool(name="projw", bufs=1) as pw,
                tc.tile_pool(name="rope", bufs=1) as pr,
                tc.tile_pool(name="wop", bufs=1) as pwo,
            ):
                wq_sb = pw.tile([128, KC, QD], BF)
                wk_sb = pw.tile([128, KC, HD], BF)
                wv_sb = pw.tile([128, KC, HD], BF)
                cos_sb = pw.tile([128, S], BF)
                sin_sb = pw.tile([128, S], BF)
                wo_sb = pwo.tile([128, GQ, H], BF)

                # All input DMAs on the SP HWDGE queue, in first-needed
                # order, sized so the ko=0 accumulation starts after ~1MB.
                x_sb = [None] * TC

                def ld_x(t, pieces):
                    xt = x_sb[t]
                    if xt is None:
                        xt = pw.tile([128, KC, 512], BF, tag="xc", bufs=2,
                                     name=f"x_sb{t}")
                        x_sb[t] = xt
                    for k0, k1 in pieces:
                        nc.sync.dma_start(
                            xt[:, k0:k1, :],
                            xP[:, 512 * (KC * t + k0):512 * (KC * t + k1)])

                def ld_wq(k0, k1):
                    nc.sync.dma_start(wq_sb[:, k0:k1, :],
                                      wqP[:, QD * k0:QD * k1])

                ld_x(0, [(0, 1)])
                ld_wq(0, 1)
                ld_x(0, [(1, 2)])
                ld_wq(1, 2)
                ld_x(0, [(2, 4)])
                ld_wq(2, 4)
                nc.sync.dma_start(wk_sb[:, :, :], wkP)
                nc.sync.dma_start(wv_sb[:, :, :], wvP)
                ld_x(0, [(4, 8)])
                ld_wq(4, 8)
                ld_x(0, [(8, 16)])
                ld_wq(8, 16)
                nc.sync.dma_start(cos_sb[:, :], cosT)
                nc.sync.dma_start(sin_sb[:, :], sinS)

                for t in range(TC):
                    ts = slice(512 * t, 512 * t + 512)
                    if t + 1 < TC:
                        ld_x(t + 1, [(0, 8), (8, 16)])
                    if t == 0:
                        nc.sync.dma_start(wo_sb[:, :, :], woP)

                    pq = [ps8.tile([128, 512], F32, tag=f"A{m}",
                                   name=f"pq{m}_{t}")
                          for m in range(GQ)]
                    pk = ps8.tile([128, 512], F32, tag="Bt", bufs=2,
                                  name=f"pk_{t}")
                    pv = ps8.tile([128, 512], F32, tag="Bt", bufs=2,
                                  name=f"pv_{t}")
                    for ko in range(KC):
                        st = (ko == 0)
                        sp = (ko == KC - 1)
                        xc = x_sb[t][:, ko, :]
                        for m in range(GQ):
                            nc.tensor.matmul(
                                pq[m][:, :],
                                wq_sb[:, ko, 128 * m:128 * m + 128],
                                xc, start=st, stop=sp)
                        nc.tensor.matmul(pk[:, :], wk_sb[:, ko, :],
                                         xc, start=st, stop=sp)
                        nc.tensor.matmul(pv[:, :], wv_sb[:, ko, :],
                                         xc, start=st, stop=sp)

                    # q/k psum evictions (+bias) on ACT first — q heads
                    # before k so the A psum banks free up one by one for
                    # the previous chunk's attention scores
                    raws = []
                    for m in [0, 1, 2, 3, GQ]:
                        raw = pr.tile([128, 512], BF, tag="raw", bufs=6,
                                      name=f"raw_{t}_{m}")
                        src_ps = pq[m][:, :] if m < GQ else pk[:, :]
                        bcol = m if m < GQ else 4
                        nc.scalar.activation(
                            raw[:, :], src_ps, AF.Identity,
                            bias=bias_sb[:, bcol:bcol + 1])
                        raws.append((m, raw))

                    # v: evict with bias, then transpose to natural layout
                    # (only needed by THIS chunk's attention, next iteration)
                    vT_t = pr.tile([128, 512], BF, tag="vT", bufs=2,
                                   name=f"vT_{t}")
                    nc.scalar.activation(vT_t[:, :], pv[:, :], AF.Identity,
                                         bias=bias_sb[:, 5:6])
                    for st4 in range(4):
                        ptr = ps8.tile([128, 128], BF, tag="Ct", bufs=2,
                                       name=f"ptr_{t}_{st4}")
                        nc.tensor.transpose(
                            ptr[:, :], vT_t[:, 128 * st4:128 * st4 + 128],
                            ident[:, :])
                        nc.scalar.copy(v_sb[t][:, st4, :], ptr[:, :])

                    # attention + o_proj for earlier chunks, emitted before
                    # this chunk's RoPE DVE chain so the attention's DVE
                    # work (exp sums, normalization) drains first; the
                    # o_proj tiles of the chunk before interleave per head
                    # as PE filler for the exp-paced score pipeline.
                    # attention(0) is deliberately delayed to t=2 so that
                    # proj(2)'s dense matmuls fill its exp-chain stalls
                    # (the small first chunk has no o_proj filler of its own).
                    if t >= 2:
                        if t == 2:
                            attention(0, qf_tiles[0])
                        qc = t - 1
                        attention(qc, qf_tiles[qc],
                                  filler=lambda h, q=qc: oproj_tile(
                                      4 * (q - 1) + h, pwo, wo_sb))

                    # RoPE for q/k on DVE in [d, tok] layout (needed only by
                    # THIS chunk's attention, one iteration later)
                    qf_t = pqf.tile([128, GQ, 512], BF, tag="qf",
                                    name=f"qf_{t}")
                    qf_tiles[t] = qf_t
                    for m, raw in raws:
                        rot = pr.tile([128, 512], BF, tag="rot", bufs=2,
                                      name=f"rot_{t}_{m}")
                        nc.vector.tensor_copy(rot[0:64, :], raw[64:128, :])
                        nc.vector.tensor_copy(rot[64:128, :], raw[0:64, :])
                        t1 = pr.tile([128, 512], BF, tag="t1", bufs=2,
                                     name=f"t1_{t}_{m}")
                        nc.vector.tensor_mul(t1[:, :], rot[:, :],
                                             sin_sb[:, ts])
                        t2 = pr.tile([128, 512], BF, tag="t2", bufs=2,
                                     name=f"t2_{t}_{m}")
                        nc.vector.tensor_mul(t2[:, :], raw[:, :],
                                             cos_sb[:, ts])
                        dst = (qf_t[:, m, :] if m < GQ else kf[t][:, :])
                        nc.vector.tensor_add(dst, t1[:, :], t2[:, :])
                qc = TC - 1
                attention(qc, qf_tiles[qc],
                          filler=lambda h: oproj_tile(4 * (qc - 1) + h,
                                                      pwo, wo_sb))
                for tt in range(4 * (TC - 1), 4 * TC):
                    oproj_tile(tt, pwo, wo_sb, tail=True,
                               last=(tt == 4 * TC - 1))


def _build():
    global _NC
    if _NC is None:
        nc = bacc.Bacc("TRN2", target_bir_lowering=False, debug=False,
                       num_devices=8)
        _emit(nc)
        nc.compile()
        _NC = nc
    return _NC


def _prep_inputs(x, wq, bq, wk, bk, wv, bv, wo, bo, cos, sin):
    """Host-side shard + layout prep. Core c = (g, b): g = c % 4, b = c // 4."""
    import ml_dtypes
    bf16 = ml_dtypes.bfloat16
    inv_sqrt_d = 1.0 / math.sqrt(HD)
    f32 = np.float32
    cosT = np.ascontiguousarray(cos.T.astype(bf16))
    sinSf = sin.T.astype(f32).copy()
    sinSf[0:HD // 2] *= -1.0
    sinS = np.ascontiguousarray(sinSf.astype(bf16))

    def pack(mT):
        """[n*128, m] -> [128, n*m]: row p = concat_n mT[n*128+p, :]."""
        n = mT.shape[0] // 128
        return np.ascontiguousarray(
            mT.reshape(n, 128, mT.shape[1]).transpose(1, 0, 2)
            .reshape(128, n * mT.shape[1]).astype(bf16))

    def pack_x(xT):
        """[H, S] -> [128, TC*KC*512]: [p, t, ko, c] = xT[ko*128+p, 512t+c]
        (token-chunk-major so each projection chunk is one contiguous run)."""
        return np.ascontiguousarray(
            xT.reshape(KC, 128, TC, 512).transpose(1, 2, 0, 3)
            .reshape(128, TC * KC * 512).astype(bf16))

    xPb = [pack_x(x[b].T.astype(f32)) for b in range(B)]

    in_maps = []
    for c in range(8):
        g, b = c % G, c // G
        wq_s = wq[QD * g:QD * (g + 1), :] * inv_sqrt_d
        bq_s = bq[QD * g:QD * (g + 1)] * inv_sqrt_d
        wk_s = wk[HD * g:HD * (g + 1), :]
        bk_s = bk[HD * g:HD * (g + 1)]
        wv_s = wv[HD * g:HD * (g + 1), :]
        bv_s = bv[HD * g:HD * (g + 1)]
        bias = np.zeros((128, 6), f32)
        bias[:, 0:4] = bq_s.reshape(GQ, HD).T
        bias[:, 4] = bk_s
        bias[:, 5] = bv_s
        in_maps.append({
            "xP": xPb[b],
            "wqP": pack(wq_s.T),
            "wkP": pack(wk_s.T),
            "wvP": pack(wv_s.T),
            "woP": pack(wo[:, QD * g:QD * (g + 1)].T),
            "cosT": cosT,
            "sinS": sinS,
            "bqkv": bias,
            "onesd": np.ones((128, 128), bf16),
        })
    return in_maps


def run(inputs, trace=False):
    """Returns (full_output, BassKernelResults)."""
    inputs = {k: np.asarray(v) for k, v in inputs.items()}
    nc = _build()
    in_maps = _prep_inputs(**inputs)
    res = run_bass_kernel_spmd(nc, in_maps, core_ids=list(range(8)),
                               trace=trace)
    bo = inputs["bo"].astype(np.float64)
    out = np.empty((B, S, H), np.float32)
    for b in range(B):
        acc = np.zeros((S, H), np.float64)
        for g in range(G):
            acc += res.results[G * b + g]["outp"].astype(np.float64)
        out[b] = (acc + bo).astype(np.float32)
    return out, res


def kernel(**inputs):
    return run(inputs, trace=False)[0]
